# revision 8
# baseline (speedup 1.0000x reference)
"""Trainium2 Bass kernel for nn_Bernstein (gnn_message_passing) — v2.

Math: the reference collapses to out[b] = sum_{k=0..5} (L^k x_b) @ U_k with
folded 64x64 U_k (incl. the stale-x3 quirk).

v2 strategy (vs v1's batch-per-core 256B gathers): shard the M=49152 rows
across the 8 cores; each core computes its 6144 rows of V_k = L V_{k-1} for
ALL 8 batches at once.  Each edge then gathers a 2KB descriptor (pair of
rows x 8 batches x 64 feats fp16) instead of 256B — per-descriptor cost on
HW is flat in size (~0.1us), so this is ~8x fewer descriptor-latencies.
Tables are exchanged between cores once per SpMM with a DRAM AllGather.
Projection accumulates out^T per core via DMA-transpose + doubled-U matmuls
(same machinery as v1), finalized with PE transposes.
"""

import numpy as np
from math import comb

B = 8
M = 49152
F = 64
KPOW = 5
N_CORES = 8

RPC = M // N_CORES          # rows per core (6144)
T = 16                      # rows per partition per supertile
ST_ROWS = 128 * T           # rows per supertile (2048)
N_ST = RPC // ST_ROWS       # supertiles per core (3)
NIDX = 128 * T              # idxs per gather call (2048) = one neighbor d
PAIRS = M // 2              # table pair-rows (24576)
SLICE_PAIRS = RPC // 2      # pair-rows per core slice (3072)
ROWB = 1024                 # table row width: [u2, b8, f64] fp16
SUB = 8                     # 128-wide sub-rows per table row
CHP = 512                   # projection chunk (sub-rows)
NCH = SLICE_PAIRS * SUB // CHP   # proj chunks per k (48)


def _theta_coeffs(deg):
    c = np.zeros((deg + 1, deg + 1), dtype=np.float64)
    for i in range(deg):
        theta = comb(deg, i) / 2 ** deg
        for j in range(deg - i + 1):
            c[i, i + j] += theta * comb(deg - i, j) * (2.0 ** (deg - i - j)) * ((-1.0) ** j)
    c[deg, :] = (comb(deg, deg) / 2 ** deg) * c[deg - 1, :]
    return c


def _host_prep(input_tensor, L_vals, kernel, L_rows, L_cols):
    """Index/layout prep + weight folding.  Returns per-core input maps."""
    nnz = L_rows.shape[0]
    cnt = np.bincount(L_rows, minlength=M)
    deg = int(cnt.max())
    if deg * M == nnz and np.all(cnt == deg):
        cols_d = L_cols.reshape(M, deg).astype(np.int64)
        vals_d = L_vals.reshape(M, deg).astype(np.float32)
    else:
        cols_d = np.zeros((M, deg), dtype=np.int64)
        vals_d = np.zeros((M, deg), dtype=np.float32)
        starts = np.concatenate([[0], np.cumsum(cnt)[:-1]])
        pos = np.arange(nnz) - starts[L_rows]
        cols_d[L_rows, pos] = L_cols
        vals_d[L_rows, pos] = L_vals

    # folded U_k [6, F, F]
    c = _theta_coeffs(KPOW)
    Wr = np.asarray(kernel, dtype=np.float64).reshape(F, KPOW + 1, F)
    U = np.einsum('ik,fio->kfo', c, Wr)                     # [6, F, F]
    # doubled block-diag for the (b2, f) partition layout
    U2 = np.zeros((KPOW + 1, 2 * F, 2 * F), dtype=np.float16)
    for k in range(KPOW + 1):
        U2[k, :F, :F] = U[k]
        U2[k, F:, F:] = U[k]

    # x table slice per core: xs[c][pr, u, b, f] = x[b, RPC*c + 2*pr + u, f]
    x = np.asarray(input_tensor, dtype=np.float32)          # [B, M, F]
    xt = np.transpose(x, (1, 0, 2)).astype(np.float16)      # [M, B, F]
    xt = xt.reshape(M // 2, 2, B, F)                        # [pr, u, b, f]
    x_slices = xt.reshape(N_CORES, SLICE_PAIRS, ROWB)

    # per-core gather idx + vals2
    # m(core, st, p, t) = RPC*core + st*ST_ROWS + p*T + t
    m_idx = (np.arange(N_CORES)[:, None, None, None] * RPC
             + np.arange(N_ST)[None, :, None, None] * ST_ROWS
             + np.arange(128)[None, None, :, None] * T
             + np.arange(T)[None, None, None, :])           # [8, 3, 128, 16]
    cols_m = cols_d[m_idx]                                  # [8, 3, 128, 16, deg]
    vals_m = vals_d[m_idx]

    # gather call (st, d): idx position j = t*128 + p -> pair col>>1, remapped
    # to the supertile-major table layout: phys(g) = (g%3072)//1024*8192
    # + (g//3072)*1024 + (g%1024)  (tables stored [st, core, 1024 rows]).
    g = cols_m >> 1
    gphys = (g % SLICE_PAIRS) // 1024 * (N_CORES * 1024) \
        + (g // SLICE_PAIRS) * 1024 + (g % 1024)
    idxs = gphys.transpose(0, 1, 4, 3, 2)                   # [8, 3, deg, 16, 128]
    idx_flat = idxs.reshape(N_CORES, N_ST, deg, NIDX)
    # wrapped [128, NIDX/16]: wrapped[p, i] = idx_flat[i*16 + p%16]
    w = idx_flat.reshape(N_CORES, N_ST, deg, NIDX // 16, 16)
    w = w.transpose(0, 1, 2, 4, 3)                          # [.., 16, NIDX/16]
    idx_w = np.tile(w, (1, 1, 1, 8, 1)).astype(np.int16)    # [8, 3, deg, 128, NIDX/16]

    # vals2 [8, 3, 128, deg, T, 2]: one-hot by col parity
    par = (cols_m & 1)                                      # [8, 3, 128, 16, deg]
    v2 = np.zeros((N_CORES, N_ST, 128, T, deg, 2), dtype=np.float32)
    np.put_along_axis(v2, par.transpose(0, 1, 2, 3, 4)[..., None],
                      vals_m[..., None], axis=-1)
    vals2 = v2.transpose(0, 1, 2, 4, 3, 5).astype(np.float16)  # [8,3,128,deg,T,2]

    return deg, U2, x_slices, idx_w, vals2


def build_program(deg, num_devices=N_CORES, repeats=1, no_ag=False,
                  no_gather=False):
    import concourse.bacc as bacc
    import concourse.tile as tile
    import concourse.mybir as mybir
    from concourse import bass
    from concourse.masks import make_identity

    fp16 = mybir.dt.float16
    fp32 = mybir.dt.float32
    i16 = mybir.dt.int16

    nc = bacc.Bacc("TRN2", target_bir_lowering=False, debug=False,
                   num_devices=num_devices, num_swdge_queues=4)

    # ---- I/O ----
    xs_in = nc.dram_tensor("x_slice", [SLICE_PAIRS, ROWB], fp16,
                           kind="ExternalInput").ap()
    idx_in = nc.dram_tensor("idx_w", [N_ST, deg, 128, NIDX // 16], i16,
                            kind="ExternalInput").ap()
    vals_in = nc.dram_tensor("vals2", [N_ST, 128, deg, T, 2], fp16,
                             kind="ExternalInput").ap()
    u2_in = nc.dram_tensor("u2", [KPOW + 1, 2 * F, 2 * F], fp16,
                           kind="ExternalInput").ap()
    out_y = nc.dram_tensor("out_y", [RPC, B * F], fp32,
                           kind="ExternalOutput").ap()

    # ---- DRAM ----
    xtab = nc.dram_tensor("xtab", [PAIRS, ROWB], fp16, kind="Internal").ap()
    xloc = nc.dram_tensor("xloc", [SLICE_PAIRS, ROWB], fp16, kind="Internal").ap()
    tabs = [nc.dram_tensor(f"tab{i}", [PAIRS, ROWB], fp16, kind="Internal").ap()
            for i in range(2)]
    slices = [nc.dram_tensor(f"slice{i}", [SLICE_PAIRS, ROWB], fp16,
                             kind="Internal").ap()
              for i in range(2)]

    RG = [list(range(num_devices))]

    with tile.TileContext(nc) as tc:
        with tc.tile_pool(name="persist", bufs=1) as pp, \
             tc.tile_pool(name="gat", bufs=3) as gp, \
             tc.tile_pool(name="ixp", bufs=6) as xp, \
             tc.tile_pool(name="sacc", bufs=2) as sp2, \
             tc.tile_pool(name="proj", bufs=3) as jp, \
             tc.tile_pool(name="psum", bufs=3, space="PSUM") as sp:

            vals_sb = pp.tile([128, N_ST, deg, T, 2], fp16)
            u2_sb = pp.tile([2 * F, KPOW + 1, 2 * F], fp16)
            ident = pp.tile([128, 128], fp16)
            acc = pp.tile([128, SLICE_PAIRS * SUB], fp16)   # out^T accumulator

            nc.sync.dma_start(out=vals_sb[:], in_=vals_in.transpose([1, 0, 2, 3, 4]))
            nc.sync.dma_start(out=u2_sb[:], in_=u2_in.transpose([1, 0, 2]))
            make_identity(nc, ident[:])

            # stage x slice into the shared x table via per-supertile AllGathers
            # (st-major table layout keeps each collective output contiguous)
            nc.sync.dma_start(out=xloc, in_=xs_in)
            xtab_v = xtab.rearrange("(s q) x -> s (q x)", s=N_ST)
            sp_rows = ST_ROWS // 2
            for st in range(N_ST):
                nc.gpsimd.collective_compute(
                    "AllGather", mybir.AluOpType.bypass, replica_groups=RG,
                    ins=[xloc[st * sp_rows:(st + 1) * sp_rows, :]],
                    outs=[xtab_v[st]])

            def proj_pass(k, src_ap):
                """acc (+)= U2_k^T @ transposed-src (local slice, [SLICE_PAIRS*SUB, 128])."""
                sub = src_ap.rearrange("r (s x) -> (r s) x", s=SUB)
                for cch in range(NCH):
                    vt = jp.tile([128, CHP], fp16, tag="vt")
                    nc.sync.dma_start(
                        out=vt[:], in_=sub[cch * CHP:(cch + 1) * CHP, :],
                        transpose=True)
                    ps = sp.tile([128, CHP], fp32, tag="mm")
                    nc.tensor.matmul(out=ps[:], lhsT=u2_sb[:, k, :], rhs=vt[:],
                                     start=True, stop=True)
                    dst = acc[:, cch * CHP:(cch + 1) * CHP]
                    if k == 0:
                        nc.any.tensor_copy(out=dst, in_=ps[:])
                    else:
                        nc.any.tensor_add(out=dst, in0=dst, in1=ps[:])

            # k = 0 projection from the local x slice
            proj_pass(0, xs_in)

            for rep in range(repeats):
                for k in range(1, KPOW + 1):
                    if k == 1:
                        src = xtab
                    else:
                        src = tabs[k % 2]
                    dst_slice = slices[k % 2]
                    dst_tab = tabs[(k + 1) % 2]
                    for st in range(N_ST):
                        S = sp2.tile([128, T, F * B], fp16, tag="S")
                        for d in range(deg if not no_gather else 0):
                            idxt = xp.tile([128, NIDX // 16], i16, tag="ix")
                            nc.sync.dma_start(out=idxt[:], in_=idx_in[st, d])
                            G = gp.tile([128, T, 2, F * B], fp16, tag="G")
                            nc.gpsimd.dma_gather(
                                G[:].rearrange("p t u x -> p t (u x)"),
                                src[:], idxt[:],
                                num_idxs=NIDX, num_idxs_reg=NIDX,
                                elem_size=ROWB, single_packet=False,
                                queue_num=d % 4)
                            nc.vector.tensor_tensor(
                                out=G[:], in0=G[:],
                                in1=vals_sb[:, st, d, :, :].unsqueeze(-1)
                                .to_broadcast([128, T, 2, F * B]),
                                op=mybir.AluOpType.mult)
                            if d == 0:
                                nc.any.tensor_add(
                                    out=S[:], in0=G[:, :, 0, :], in1=G[:, :, 1, :])
                            else:
                                nc.any.tensor_add(
                                    out=S[:], in0=S[:], in1=G[:, :, 0, :])
                                nc.any.tensor_add(
                                    out=S[:], in0=S[:], in1=G[:, :, 1, :])
                        if no_gather:
                            nc.vector.memset(S[:], 0.5)
                        # store supertile: pair-rows st*1024 + p*8 + j
                        nc.sync.dma_start(
                            out=dst_slice[st * (ST_ROWS // 2):(st + 1) * (ST_ROWS // 2), :]
                            .rearrange("(p j) x -> p (j x)", p=128),
                            in_=S[:])
                        if k < KPOW and not no_ag:
                            # sub-AllGather this supertile's slice rows into the
                            # next table (st-major: contiguous block per st),
                            # overlapping the next supertile's gathers.
                            tab_v = dst_tab.rearrange(
                                "(s q) x -> s (q x)", s=N_ST)
                            nc.gpsimd.collective_compute(
                                "AllGather", mybir.AluOpType.bypass,
                                replica_groups=RG,
                                ins=[dst_slice[st * sp_rows:(st + 1) * sp_rows, :]],
                                outs=[tab_v[st]])
                    # projection of V_k from the local slice (overlaps AG + next k)
                    proj_pass(k, dst_slice)

            # ---- finalize: acc [128=(b2,fo), (pr, w)] -> out rows ----
            for j in range(SLICE_PAIRS * SUB // 128):
                pt = sp.tile([128, 128], fp16, tag="tr")
                nc.tensor.transpose(out=pt[:], in_=acc[:, j * 128:(j + 1) * 128],
                                    identity=ident[:])
                stl = jp.tile([128, 128], fp32, tag="st")
                nc.any.tensor_copy(out=stl[:], in_=pt[:])
                # sub-row global = j*128 + (pr16, u2, bp4); value cols (b2, fo)
                # out_y flat row m=2pr+u, col (2bp+b2)*64+fo
                orows = out_y.rearrange(
                    "(q pr u) (bp b2 f) -> q (pr u bp) (b2 f)",
                    pr=16, u=2, bp=4, b2=2)
                nc.sync.dma_start(out=orows[j], in_=stl[:])
    nc.compile()
    return nc


_cache = {}


def _get_program(deg):
    if deg not in _cache:
        _cache[deg] = build_program(deg)
    return _cache[deg]


def kernel(input_tensor, L_vals, kernel, L_rows, L_cols):
    from concourse import bass_utils

    deg, U2, x_slices, idx_w, vals2 = _host_prep(
        np.asarray(input_tensor), np.asarray(L_vals), np.asarray(kernel),
        np.asarray(L_rows), np.asarray(L_cols))
    nc = _get_program(deg)

    in_maps = []
    for c in range(N_CORES):
        in_maps.append({
            "x_slice": np.ascontiguousarray(x_slices[c]),
            "idx_w": np.ascontiguousarray(idx_w[c]),
            "vals2": np.ascontiguousarray(vals2[c]),
            "u2": U2,
        })
    res = bass_utils.run_bass_kernel_spmd(nc, in_maps, core_ids=list(range(N_CORES)))
    # out_y [RPC, B*F] per core; rows = core*RPC + r, cols = (b, f)
    out = np.empty((B, M, F), dtype=np.float32)
    for c in range(N_CORES):
        o = res.results[c]["out_y"].reshape(RPC, B, F)
        out[:, c * RPC:(c + 1) * RPC, :] = o.transpose(1, 0, 2)
    return out
